# revision 4
# baseline (speedup 1.0000x reference)
"""Trainium2 Bass kernel for CrossAttention with per-head QK LayerNorm.

Sharding: 8 cores = 2 batches x 4 head-groups (4 heads of Hd=128 each).
Each core computes its heads' q/k/v projections, per-head QK layernorm,
transposed attention scores, softmax, attention output, and a partial
output projection. Host sums the 4 per-batch partials and reassembles
attn_weights from the per-core [h, s, t] layout as a transpose view.
"""

import contextlib

import numpy as np
import ml_dtypes

import concourse.bacc as bacc
import concourse.tile as tile
from concourse import mybir
from concourse.bass_utils import run_bass_kernel_spmd

F32 = mybir.dt.float32
BF16 = mybir.dt.bfloat16
AF = mybir.ActivationFunctionType
ALU = mybir.AluOpType

N_CORES = 8
EPS = 1e-6


def build_nc(Lq=2048, Lkv=4096, D=2048, HC=4):
    """Build the per-core program. HC = heads per core (Hd=128 each)."""
    Hd = 128
    J = HC * Hd            # this core's head-dim span (512)
    KK = D // 128          # contraction tiles
    TT = Lq // 128         # q time tiles
    ST = Lkv // 128        # kv s tiles
    TGW = min(512, Lq)     # t-group width
    TG = Lq // TGW
    scale = float(Hd) ** -0.5

    nc = bacc.Bacc("TRN2", target_bir_lowering=False, debug=False)

    xt = nc.dram_tensor("xt", [D, Lq], BF16, kind="ExternalInput").ap()
    ct = nc.dram_tensor("ct", [D, Lkv], BF16, kind="ExternalInput").ap()
    wq = nc.dram_tensor("wq", [D, J], BF16, kind="ExternalInput").ap()
    wk = nc.dram_tensor("wk", [D, J], BF16, kind="ExternalInput").ap()
    wv = nc.dram_tensor("wv", [D, J], BF16, kind="ExternalInput").ap()
    wo = nc.dram_tensor("wo", [J, D], BF16, kind="ExternalInput").ap()
    qbrow = nc.dram_tensor("qbrow", [1, J], BF16, kind="ExternalInput").ap()
    kbrow = nc.dram_tensor("kbrow", [1, J], BF16, kind="ExternalInput").ap()
    vbrow = nc.dram_tensor("vbrow", [1, J], BF16, kind="ExternalInput").ap()
    obrow = nc.dram_tensor("obrow", [1, D], BF16, kind="ExternalInput").ap()
    ones1_d = nc.dram_tensor("ones1", [1, 128], BF16, kind="ExternalInput").ap()
    onesg_d = nc.dram_tensor("onesg", [1, 512], BF16, kind="ExternalInput").ap()
    ones128_d = nc.dram_tensor("ones128", [128, 1], BF16, kind="ExternalInput").ap()
    ident_d = nc.dram_tensor("ident", [128, 128], BF16, kind="ExternalInput").ap()
    qg_d = nc.dram_tensor("qg", [128, 1], F32, kind="ExternalInput").ap()
    qb_d = nc.dram_tensor("qb", [128, 1], F32, kind="ExternalInput").ap()
    kg_d = nc.dram_tensor("kg", [128, 1], F32, kind="ExternalInput").ap()
    kb_d = nc.dram_tensor("kb", [128, 1], F32, kind="ExternalInput").ap()

    p_out = nc.dram_tensor("p_out", [HC, Lkv, Lq], F32, kind="ExternalOutput").ap()
    o_out = nc.dram_tensor("o_out", [Lq, D], F32, kind="ExternalOutput").ap()

    with tile.TileContext(nc) as tc:
        with contextlib.ExitStack() as stack:
            const = stack.enter_context(tc.tile_pool(name="const", bufs=1))
            ones1 = const.tile([1, 128], BF16)
            nc.sync.dma_start(ones1[:], ones1_d[:])
            onesg = const.tile([1, 512], BF16)
            nc.sync.dma_start(onesg[:], onesg_d[:])
            ones128 = const.tile([128, 1], BF16)
            nc.sync.dma_start(ones128[:], ones128_d[:])
            ident = const.tile([128, 128], BF16)
            nc.sync.dma_start(ident[:], ident_d[:])
            qg = const.tile([128, 1], F32)
            nc.sync.dma_start(qg[:], qg_d[:])
            qb = const.tile([128, 1], F32)
            nc.sync.dma_start(qb[:], qb_d[:])
            kg = const.tile([128, 1], F32)
            nc.sync.dma_start(kg[:], kg_d[:])
            kb = const.tile([128, 1], F32)
            nc.sync.dma_start(kb[:], kb_d[:])
            qbr = const.tile([1, J], BF16)
            nc.sync.dma_start(qbr[:], qbrow[:])
            kbr = const.tile([1, J], BF16)
            nc.sync.dma_start(kbr[:], kbrow[:])
            vbr = const.tile([1, J], BF16)
            nc.sync.dma_start(vbr[:], vbrow[:])
            obr = const.tile([1, D], BF16)
            nc.sync.dma_start(obr[:], obrow[:])
            eps128 = const.tile([128, 1], F32)
            nc.vector.memset(eps128[:], EPS)

            persist = stack.enter_context(tc.tile_pool(name="persist", bufs=1))
            qT = [persist.tile([128, Lq], BF16, name=f"qT{h}", tag=f"qT{h}") for h in range(HC)]
            kT = [persist.tile([128, Lkv], BF16, name=f"kT{h}", tag=f"kT{h}") for h in range(HC)]
            vv = [persist.tile([128, Lkv], BF16, name=f"vv{h}", tag=f"vv{h}") for h in range(HC)]
            wo_sb = [persist.tile([128, D], BF16, name=f"wo{h}", tag=f"wo{h}") for h in range(HC)]
            for h in range(HC):
                nc.sync.dma_start(wo_sb[h][:], wo[h * 128:(h + 1) * 128, :])

            stat = stack.enter_context(tc.tile_pool(name="stat", bufs=3))

            def ln_block(psum_pool, ps_tile, out_cb):
                """LN over per-head 128-blocks of the free dim of natural
                [t/s 128, J] psum; hand bf16-normalized per-head slices to
                out_cb for transposition."""
                sums4 = stat.tile([128, HC], F32, tag="sums4")
                nc.vector.tensor_reduce(
                    sums4[:], ps_tile[:].rearrange("p (h j) -> p h j", j=128),
                    axis=mybir.AxisListType.X, op=ALU.add,
                )
                sq = stat.tile([128, J], F32, tag="sq")
                nc.scalar.activation(sq[:], ps_tile[:], AF.Square)
                sq4 = stat.tile([128, HC], F32, tag="sq4")
                nc.vector.tensor_reduce(
                    sq4[:], sq[:].rearrange("p (h j) -> p h j", j=128),
                    axis=mybir.AxisListType.X, op=ALU.add,
                )
                m2 = stat.tile([128, HC], F32, tag="m2")
                nc.vector.tensor_tensor(m2[:], sums4[:], sums4[:], op=ALU.mult)
                v128 = stat.tile([128, HC], F32, tag="v128")
                nc.vector.scalar_tensor_tensor(
                    v128[:], m2[:], -1.0 / 128.0, sq4[:],
                    op0=ALU.mult, op1=ALU.add,
                )  # = 128*var
                rs = stat.tile([128, HC], F32, tag="rs")
                nc.scalar.activation(rs[:], v128[:], AF.Ln, scale=1.0 / 128.0, bias=eps128[:])
                nc.scalar.activation(rs[:], rs[:], AF.Exp, scale=-0.5)
                negcm = stat.tile([128, HC], F32, tag="negcm")
                nc.vector.tensor_tensor(negcm[:], sums4[:], rs[:], op=ALU.mult)
                nc.vector.tensor_scalar_mul(negcm[:], negcm[:], -1.0 / 128.0)
                lnt = stat.tile([128, J], BF16, tag="lnt")
                for h in range(HC):
                    nc.vector.tensor_scalar(
                        lnt[:, h * 128:(h + 1) * 128],
                        ps_tile[:, h * 128:(h + 1) * 128],
                        rs[:, h:h + 1], negcm[:, h:h + 1],
                        op0=ALU.mult, op1=ALU.add,
                    )
                for h in range(HC):
                    out_cb(h, lnt[:, h * 128:(h + 1) * 128])

            # ---------------- Phase Q (two t-halves) ----------------
            TH = Lq // 2
            for th in range(2):
                with tc.tile_pool(name=f"xpool{th}", bufs=1) as xpool, \
                     tc.tile_pool(name=f"wqpool{th}", bufs=1) as wqpool, \
                     tc.tile_pool(name=f"psq{th}", bufs=1, space="PSUM") as psum_q:
                    xt_sb = [xpool.tile([128, TH], BF16, name=f"xt{th}_{kk}", tag=f"xt{kk}") for kk in range(KK)]
                    wq_sb = [wqpool.tile([128, J], BF16, name=f"wq{th}_{kk}", tag=f"wq{kk}") for kk in range(KK)]
                    for kk in range(KK):
                        nc.sync.dma_start(
                            xt_sb[kk][:], xt[kk * 128:(kk + 1) * 128, th * TH:(th + 1) * TH])
                        nc.sync.dma_start(wq_sb[kk][:], wq[kk * 128:(kk + 1) * 128, :])
                    psTq = {}
                    for tl in range(TH // 128):
                        ti = th * (TH // 128) + tl
                        psq = psum_q.tile([128, J], F32, tag="pproj", name=f"psq{ti}", bufs=2)
                        for kk in range(KK):
                            nc.tensor.matmul(
                                psq[:], xt_sb[kk][:, tl * 128:(tl + 1) * 128],
                                wq_sb[kk][:], start=(kk == 0), stop=False,
                            )
                        nc.tensor.matmul(psq[:], ones1[:], qbr[:], start=False, stop=True)

                        def q_emit(h, lnt_slice, ti=ti, psum_q=psum_q):
                            if ti % 4 == 0:
                                psTq[h] = psum_q.tile(
                                    [128, 512], BF16, tag=f"ptr{h}", name=f"psTq{h}_{ti}")
                            nc.tensor.transpose(
                                psTq[h][:, (ti % 4) * 128:(ti % 4 + 1) * 128],
                                lnt_slice, ident[:],
                            )
                            if ti % 4 == 3:
                                nc.vector.tensor_scalar(
                                    qT[h][:, (ti // 4) * 512:(ti // 4 + 1) * 512],
                                    psTq[h][:], qg[:], qb[:], op0=ALU.mult, op1=ALU.add,
                                )

                        ln_block(psum_q, psq, q_emit)

            # ---------------- Phase KV (four s-quarters) ----------------
            with tc.tile_pool(name="wkvpool", bufs=1) as wkvpool:
                wk_sb = [wkvpool.tile([128, J], BF16, name=f"wk{kk}", tag=f"wk{kk}") for kk in range(KK)]
                wv_sb = [wkvpool.tile([128, J], BF16, name=f"wv{kk}", tag=f"wv{kk}") for kk in range(KK)]
                for kk in range(KK):
                    nc.sync.dma_start(wk_sb[kk][:], wk[kk * 128:(kk + 1) * 128, :])
                    nc.sync.dma_start(wv_sb[kk][:], wv[kk * 128:(kk + 1) * 128, :])
                SH = Lkv // 4
                for quarter in range(4):
                    with tc.tile_pool(name=f"cpool{quarter}", bufs=1) as cpool:
                        ct_sb = [cpool.tile([128, SH], BF16, name=f"ct{quarter}_{kk}", tag=f"ct{kk}") for kk in range(KK)]
                        for kk in range(KK):
                            nc.sync.dma_start(
                                ct_sb[kk][:], ct[kk * 128:(kk + 1) * 128,
                                               quarter * SH:(quarter + 1) * SH])
                        with tc.tile_pool(name=f"psk{quarter}", bufs=1, space="PSUM") as psum_k:
                            psTk = {}
                            for sl in range(SH // 128):
                                sg = quarter * (SH // 128) + sl
                                psk = psum_k.tile([128, J], F32, tag="pproj", name=f"psk{sg}", bufs=2)
                                for kk in range(KK):
                                    nc.tensor.matmul(
                                        psk[:], ct_sb[kk][:, sl * 128:(sl + 1) * 128],
                                        wk_sb[kk][:], start=(kk == 0), stop=False,
                                    )
                                nc.tensor.matmul(psk[:], ones1[:], kbr[:], start=False, stop=True)

                                def k_emit(h, lnt_slice, sg=sg, psum_k=psum_k):
                                    if sg % 4 == 0:
                                        psTk[h] = psum_k.tile(
                                            [128, 512], BF16, tag=f"ptr{h}", name=f"psTk{h}_{sg}")
                                    nc.tensor.transpose(
                                        psTk[h][:, (sg % 4) * 128:(sg % 4 + 1) * 128],
                                        lnt_slice, ident[:],
                                    )
                                    if sg % 4 == 3:
                                        nc.vector.tensor_scalar(
                                            kT[h][:, (sg // 4) * 512:(sg // 4 + 1) * 512],
                                            psTk[h][:], kg[:], kb[:], op0=ALU.mult, op1=ALU.add,
                                        )

                                ln_block(psum_k, psk, k_emit)
                        with tc.tile_pool(name=f"psv{quarter}", bufs=1, space="PSUM") as psum_v:
                            for sgl in range(SH // 512):
                                for h in range(HC):
                                    psv = psum_v.tile([128, 512], F32, tag="pprojv",
                                                      name=f"psv{quarter}_{sgl}_{h}", bufs=2)
                                    for kk in range(KK):
                                        nc.tensor.matmul(
                                            psv[:], wv_sb[kk][:, h * 128:(h + 1) * 128],
                                            ct_sb[kk][:, sgl * 512:(sgl + 1) * 512],
                                            start=(kk == 0), stop=False,
                                        )
                                    nc.tensor.matmul(
                                        psv[:], vbr[:, h * 128:(h + 1) * 128], onesg[:],
                                        start=False, stop=True,
                                    )
                                    vts = stat.tile([128, 512], BF16, tag="vts")
                                    nc.vector.tensor_copy(vts[:], psv[:])
                                    psvt = psum_v.tile([128, 512], BF16, tag="ptrv",
                                                       name=f"psvt{quarter}_{sgl}_{h}", bufs=2)
                                    for i in range(4):
                                        nc.tensor.transpose(
                                            psvt[:, i * 128:(i + 1) * 128],
                                            vts[:, i * 128:(i + 1) * 128], ident[:],
                                        )
                                    s0 = quarter * SH + sgl * 512
                                    nc.vector.tensor_copy(vv[h][:, s0:s0 + 512], psvt[:])

            # ---------------- Phase ATT (per head) ----------------
            with tc.tile_pool(name="expp", bufs=1) as expp, \
                 tc.tile_pool(name="pp", bufs=1) as pp, \
                 tc.tile_pool(name="persist2", bufs=1) as persist2, \
                 tc.tile_pool(name="psatt", bufs=1, space="PSUM") as psum_a:
                aT = [persist2.tile([128, Lq], BF16, name=f"aT{h}", tag=f"aT{h}") for h in range(HC)]
                for h in range(HC):
                    for tg in range(TG):
                        t0 = tg * TGW
                        psO = psum_a.tile([128, TGW], F32, tag="psO", name=f"psO{h}_{tg}")
                        psSum = psum_a.tile([1, TGW], F32, tag="psSum", name=f"psSum{h}_{tg}")
                        exps = []
                        for st in range(ST):
                            psS = psum_a.tile([128, TGW], F32, tag="psS",
                                              name=f"psS{h}_{tg}_{st}", bufs=3)
                            nc.tensor.matmul(
                                psS[:], kT[h][:, st * 128:(st + 1) * 128],
                                qT[h][:, t0:t0 + TGW], start=True, stop=True,
                            )
                            eb = expp.tile([128, TGW], BF16, tag="expsb",
                                           name=f"eb{h}_{tg}_{st}", bufs=ST + 2)
                            nc.scalar.activation(eb[:], psS[:], AF.Exp, scale=scale)
                            exps.append(eb)
                            nc.tensor.matmul(
                                psSum[:], ones128[:], eb[:],
                                start=(st == 0), stop=(st == ST - 1),
                            )
                            nc.tensor.matmul(
                                psO[:], vv[h][:, st * 128:(st + 1) * 128], eb[:],
                                start=(st == 0), stop=(st == ST - 1),
                            )
                        # recip of sums: exp(-ln(x))
                        rrow = stat.tile([1, TGW], F32, tag="rrow")
                        nc.scalar.activation(rrow[:], psSum[:], AF.Ln)
                        nc.scalar.activation(rrow[:], rrow[:], AF.Exp, scale=-1.0)
                        rrow_bf = stat.tile([1, TGW], BF16, tag="rrowbf")
                        nc.vector.tensor_copy(rrow_bf[:], rrow[:])
                        psRB = psum_a.tile([128, TGW], F32, tag="psRB", name=f"psRB{h}_{tg}")
                        nc.tensor.matmul(psRB[:], ones1[:], rrow_bf[:], start=True, stop=True)
                        rb_bf = stat.tile([128, TGW], BF16, tag="rbbf")
                        nc.vector.tensor_copy(rb_bf[:], psRB[:])
                        rb_f = stat.tile([128, TGW], F32, tag="rbf")
                        nc.scalar.activation(rb_f[:], psRB[:], AF.Copy)
                        # fold recip into attention-output copy (aT)
                        nc.vector.tensor_tensor(
                            aT[h][:, t0:t0 + TGW], psO[:], rb_f[:], op=ALU.mult,
                        )
                        # normalize attn weights and store
                        for st in range(ST):
                            pt = pp.tile([128, TGW], F32, tag="ptile",
                                         name=f"pt{h}_{tg}_{st}", bufs=4)
                            nc.gpsimd.tensor_tensor(pt[:], exps[st][:], rb_bf[:], op=ALU.mult)
                            nc.sync.dma_start(
                                p_out[h, st * 128:(st + 1) * 128, t0:t0 + TGW], pt[:],
                            )

                # ---------------- Phase OPROJ ----------------
                with tc.tile_pool(name="oout", bufs=4) as oout, \
                     tc.tile_pool(name="pso_p", bufs=1, space="PSUM") as psum_o:
                    for ti in range(TT):
                        for dg in range(D // 512):
                            pso = psum_o.tile([128, 512], F32, tag="pso",
                                              name=f"pso{ti}_{dg}", bufs=2)
                            for h in range(HC):
                                nc.tensor.matmul(
                                    pso[:], aT[h][:, ti * 128:(ti + 1) * 128],
                                    wo_sb[h][:, dg * 512:(dg + 1) * 512],
                                    start=(h == 0), stop=False,
                                )
                            nc.tensor.matmul(
                                pso[:], ones1[:], obr[:, dg * 512:(dg + 1) * 512],
                                start=False, stop=True,
                            )
                            osb = oout.tile([128, 512], F32, tag="osb")
                            nc.vector.tensor_copy(osb[:], pso[:])
                            nc.sync.dma_start(
                                o_out[ti * 128:(ti + 1) * 128, dg * 512:(dg + 1) * 512],
                                osb[:],
                            )

    nc.compile()
    return nc


_NC_CACHE = {}


def _get_nc():
    if "nc" not in _NC_CACHE:
        _NC_CACHE["nc"] = build_nc()
    return _NC_CACHE["nc"]


def _bf(x):
    return np.asarray(x, np.float32).astype(ml_dtypes.bfloat16)


def make_in_maps(hidden_states, cross_attention_states, q_w, q_b, k_w, k_b,
                 v_w, v_b, o_w, o_b, qn_g, qn_b, kn_g, kn_b,
                 HC=4, n_cores=N_CORES):
    J = HC * 128
    B = hidden_states.shape[0]
    D = hidden_states.shape[2]
    n_hg = n_cores // B
    in_maps = []
    for c in range(n_cores):
        b, hg = c // n_hg, c % n_hg
        rows = slice(hg * J, (hg + 1) * J)
        m = {
            "xt": _bf(np.ascontiguousarray(hidden_states[b].T)),
            "ct": _bf(np.ascontiguousarray(cross_attention_states[b].T)),
            "wq": _bf(np.ascontiguousarray(q_w[rows].T)),
            "wk": _bf(np.ascontiguousarray(k_w[rows].T)),
            "wv": _bf(np.ascontiguousarray(v_w[rows].T)),
            "wo": _bf(np.ascontiguousarray(o_w[:, rows].T)),
            "qbrow": _bf(np.asarray(q_b)[rows][None, :]),
            "kbrow": _bf(np.asarray(k_b)[rows][None, :]),
            "vbrow": _bf(np.asarray(v_b)[rows][None, :]),
            "obrow": _bf(np.asarray(o_b)[None, :]) if hg == 0 else _bf(np.zeros((1, D))),
            "ones1": _bf(np.ones((1, 128))),
            "onesg": _bf(np.ones((1, 512))),
            "ones128": _bf(np.ones((128, 1))),
            "ident": _bf(np.eye(128)),
            "qg": np.asarray(qn_g, np.float32).reshape(128, 1).copy(),
            "qb": np.asarray(qn_b, np.float32).reshape(128, 1).copy(),
            "kg": np.asarray(kn_g, np.float32).reshape(128, 1).copy(),
            "kb": np.asarray(kn_b, np.float32).reshape(128, 1).copy(),
        }
        in_maps.append(m)
    return in_maps


def kernel(hidden_states, cross_attention_states, q_w, q_b, k_w, k_b,
           v_w, v_b, o_w, o_b, qn_g, qn_b, kn_g, kn_b):
    hidden_states = np.asarray(hidden_states, np.float32)
    cross_attention_states = np.asarray(cross_attention_states, np.float32)
    B, Lq, D = hidden_states.shape
    Lkv = cross_attention_states.shape[1]
    H = 16
    HC = 4
    n_hg = N_CORES // B

    nc = _get_nc()
    in_maps = make_in_maps(
        hidden_states, cross_attention_states, q_w, q_b, k_w, k_b,
        v_w, v_b, o_w, o_b, qn_g, qn_b, kn_g, kn_b,
    )
    res = run_bass_kernel_spmd(nc, in_maps, core_ids=list(range(N_CORES)))

    attn_output = np.zeros((B, Lq, D), np.float32)
    attn_w = np.empty((B, H, Lkv, Lq), np.float32)
    for c in range(N_CORES):
        b, hg = c // n_hg, c % n_hg
        attn_output[b] += res.results[c]["o_out"]
        attn_w[b, hg * HC:(hg + 1) * HC] = res.results[c]["p_out"]
    return attn_output, attn_w.transpose(0, 1, 3, 2)
